# revision 33
# baseline (speedup 1.0000x reference)
"""MoE gate kernel (logits -> softmax -> top-8 + aux loss) for 8 trn2 cores.

Contract: kernel(hidden_states=[2,8192,4096] f32, weight=[64,4096] f32) ->
(topk_idx [16384,8] i32, topk_weight [16384,8] f32, aux_loss scalar f32),
matching reference.py (jax) semantics.

Sharding: data-parallel over tokens (2048/core); the host pre-transposes
each shard to [D, NS] so the contraction dim lands on SBUF partitions, and
pre-splits fp32 into bf16 hi/lo planes (packed [2, D, NS], same 32MB/core
DMA traffic as fp32).

x = xh + xl (both bf16), w = wh + wl (both bf16); logits ~= wh.T@xh +
wl.T@xh + wh.T@xl (the dropped xl.wl term is ~2^-18 relative). bf16
products are exact in fp32 accumulation, so the logit error is ~4e-6
absolute vs ~1.7e-4 for fp32r — near-fp32 top-k fidelity at 1 cycle/row
matmul throughput (vs 4 for fp32), keeping the kernel DMA-bound.

Same I/O contract and sharding as kernel.py; inputs are pre-split on the
host into a packed [2, D, NS] bf16 tensor per core (same 32MB/core DMA
traffic as fp32).
"""

import sys

import numpy as np

if "/opt/trn_rl_repo" not in sys.path:
    sys.path.insert(0, "/opt/trn_rl_repo")

import ml_dtypes

BF16 = np.dtype(ml_dtypes.bfloat16)

B, S, D, E = 2, 8192, 4096, 64
N = B * S
N_CORES = 8
NS = N // N_CORES  # 2048 tokens per core
TOP_K = 8
ALPHA = 0.01
CHUNK = 512
DC = D // 128  # 32 contraction chunks
QS = 8  # d-chunks per xt DMA
NQ = DC // QS  # 4 quarter-DMAs per plane per chunk
OW = E + 2 * TOP_K  # 80 packed output row

_CACHED_NC = None


def _build_nc():
    from contextlib import ExitStack

    import concourse.mybir as mybir
    import concourse.tile as tile
    from concourse import bacc
    from concourse.masks import make_identity

    f32 = mybir.dt.float32
    bf16 = mybir.dt.bfloat16
    u32 = mybir.dt.uint32

    nc = bacc.Bacc(
        "TRN2", target_bir_lowering=False, debug=False, num_devices=N_CORES
    )

    xt = nc.dram_tensor("xt", [2, D, NS], bf16, kind="ExternalInput").ap()
    wt = nc.dram_tensor("wt", [2, D, E], bf16, kind="ExternalInput").ap()
    out_all = nc.dram_tensor("out_all", [NS, OW], f32, kind="ExternalOutput").ap()

    NCHUNK = NS // CHUNK  # 4
    TPC = CHUNK // 128  # 4

    # DRAM views with the partition dim innermost
    xt_v = xt.rearrange("h (c p) n -> h p c n", p=128)  # [2, 128, DC, NS]
    out_v = out_all.rearrange("(j t p) e -> j p t e", j=NCHUNK, p=128)

    with tile.TileContext(nc) as tc:
        with ExitStack() as ctx:
            const_pool = ctx.enter_context(tc.tile_pool(name="const", bufs=1))
            xh_pool = ctx.enter_context(tc.tile_pool(name="xh", bufs=6))
            xl_pool = ctx.enter_context(tc.tile_pool(name="xl", bufs=6))
            lt_pool = ctx.enter_context(tc.tile_pool(name="lt", bufs=2))
            stage_pool = ctx.enter_context(tc.tile_pool(name="stage", bufs=3))
            small_pool = ctx.enter_context(tc.tile_pool(name="small", bufs=8))
            psum_lt = ctx.enter_context(
                tc.tile_pool(name="psum_lt", bufs=4, space="PSUM")
            )
            psum_tr = ctx.enter_context(
                tc.tile_pool(name="psum_tr", bufs=4, space="PSUM")
            )

            ident = const_pool.tile([64, 64], f32)
            make_identity(nc, ident[:])

            # both weight planes resident in SBUF: [128, 2, DC, E] bf16
            wt_sb = const_pool.tile([128, 2, DC, E], bf16)
            nc.sync.dma_start(
                wt_sb[:], wt.rearrange("h (c p) e -> p h c e", p=128)
            )

            for j in range(NCHUNK):
                lt_psum = psum_lt.tile([64, CHUNK], f32)
                xh = [None] * NQ
                xl = [None] * NQ
                nmm = 0
                for q in range(NQ):
                    xh[q] = xh_pool.tile([128, QS, CHUNK], bf16, tag="xh", name="xh")
                    nc.sync.dma_start(
                        xh[q][:],
                        xt_v[0, :, q * QS : (q + 1) * QS,
                             j * CHUNK : (j + 1) * CHUNK],
                    )
                    xl[q] = xl_pool.tile([128, QS, CHUNK], bf16, tag="xl", name="xl")
                    nc.sync.dma_start(
                        xl[q][:],
                        xt_v[1, :, q * QS : (q + 1) * QS,
                             j * CHUNK : (j + 1) * CHUNK],
                    )
                NMM = DC * 3
                for c in range(DC):
                    q, r = divmod(c, QS)
                    # wh.T@xh + wl.T@xh + wh.T@xl accumulate into one bank
                    for h, xop in ((0, xh[q]), (1, xh[q]), (0, xl[q])):
                        nc.tensor.matmul(
                            lt_psum[:],
                            wt_sb[:, h, c, :],
                            xop[:, r, :],
                            start=(nmm == 0),
                            stop=(nmm == NMM - 1),
                        )
                        nmm += 1
                lt_sb = lt_pool.tile([64, CHUNK], f32)
                nc.scalar.copy(lt_sb[:], lt_psum[:])

                st = stage_pool.tile([128, TPC, OW], f32, tag="st")
                st_u32 = st[:].bitcast(u32)
                for i in range(TPC):
                    tr = psum_tr.tile([128, E], f32)
                    nc.tensor.transpose(
                        tr[:], lt_sb[:, i * 128 : (i + 1) * 128], ident[:]
                    )
                    sc = st[:, i, 0:E]
                    den = small_pool.tile([128, 1], f32, tag="den")
                    nc.scalar.activation(
                        sc,
                        tr[:],
                        mybir.ActivationFunctionType.Exp,
                        accum_out=den[:],
                    )
                    rden = small_pool.tile([128, 1], f32, tag="rden")
                    nc.vector.reciprocal(rden[:], den[:])
                    nc.vector.tensor_scalar_mul(sc, sc, rden[:])
                    nc.vector.max(out=st[:, i, E : E + TOP_K], in_=sc)
                    nc.vector.max_index(
                        out=st_u32[:, i, E + TOP_K : OW],
                        in_max=st[:, i, E : E + TOP_K],
                        in_values=sc,
                    )

                nc.sync.dma_start(out_v[j], st[:])
    nc.compile()
    return nc


def _get_nc():
    global _CACHED_NC
    if _CACHED_NC is None:
        _CACHED_NC = _build_nc()
    return _CACHED_NC


def _split_hi_lo(a):
    hi = a.astype(BF16)
    lo = (a - hi.astype(np.float32)).astype(BF16)
    return hi, lo


def _shard_inputs(hidden_states, weight):
    x = np.ascontiguousarray(hidden_states, dtype=np.float32).reshape(N, D)
    w = np.asarray(weight, dtype=np.float32)
    wh, wl = _split_hi_lo(np.ascontiguousarray(w.T))  # [D, E]
    wt_host = np.ascontiguousarray(np.stack([wh, wl], axis=0))  # [2, D, E]
    in_maps = []
    for m in range(N_CORES):
        xs = np.ascontiguousarray(x[m * NS : (m + 1) * NS, :].T)  # [D, NS]
        xh, xl = _split_hi_lo(xs)
        xt_host = np.ascontiguousarray(np.stack([xh, xl], axis=0))
        in_maps.append({"xt": xt_host, "wt": wt_host})
    return in_maps


def _assemble(results):
    out = np.concatenate([r["out_all"] for r in results], axis=0)  # [N, 80]
    scores = np.ascontiguousarray(out[:, :E])
    topk_w = np.ascontiguousarray(out[:, E : E + TOP_K])
    topk_i = np.ascontiguousarray(out[:, E + TOP_K :]).view(np.int32)

    pi = (scores.sum(axis=0, dtype=np.float64) / np.float64(N)).astype(np.float32)
    counts = np.bincount(
        topk_i.reshape(-1).astype(np.int64), minlength=E
    ).astype(np.float32)
    ce = counts / np.float32(N * TOP_K)
    aux_loss = np.float32(
        np.sum(pi.astype(np.float64) * ce.astype(np.float64)) * E * ALPHA
    )
    return topk_i, topk_w, aux_loss


def kernel(hidden_states, weight, _profile=False, _trace_cores=None):
    from concourse.bass_utils import run_bass_kernel_spmd

    nc = _get_nc()
    in_maps = _shard_inputs(hidden_states, weight)
    res = run_bass_kernel_spmd(nc, in_maps, list(range(N_CORES)))
    out = _assemble(res.results)
    if _profile:
        return out, res
    return out
